# revision 13
# baseline (speedup 1.0000x reference)
"""Day-routed adapter MLP (per-sample day-specific 2-layer MLP + LayerNorm)
for 8 Trainium2 NeuronCores.

Computation per sample b (day d = day_indices[b]):
    h = relu(x[b] @ W1[d] + b1[d])        # [T, D_hid]
    y = h @ W2[d] + b2[d]                 # [T, D_out]
    out = LN(y) * gamma[d] + beta[d]      # LN over last dim

Sharding: data-parallel over batch, 8 samples per core. The per-sample
day weights are gathered on the host (routing is host-visible), and x is
pre-transposed on the host so the device needs no transposes at all:

  pass 1:  hT[h_chunk, :T] += W1[k_chunk, h_chunk].T @ xT[k_chunk, :T]
           (lhsT = W1 natural layout, rhs = xT)  -> hT with H on partitions,
           so b1 is a per-partition bias fused into the ReLU copyback (ACT).
  pass 2:  y[t_tile, :O]  += hT[k_chunk, t_tile].T @ W2[k_chunk, :O]
           (lhsT = hT from pass 1, rhs = W2 natural layout) -> y with T on
           partitions and O on the free axis, which is exactly the layout
           LayerNorm wants (bn_stats/bn_aggr reduce along free axis).
"""

import os

import numpy as np
import ml_dtypes

import concourse.bass as bass
import concourse.mybir as mybir
import concourse.tile as tile
from concourse import bacc
from concourse.bass_utils import run_bass_kernel_spmd

N_CORES = 8
B, T, D_IN = 64, 512, 512
D_HID, D_OUT = 1024, 512
S = B // N_CORES  # samples per core
EPS = 1e-5

P = 128
KD = D_IN // P   # 4 contraction chunks in pass 1
KH = D_HID // P  # 8 contraction chunks in pass 2 (= H chunks of pass 1 out)
MT = T // P      # 4 token tiles in pass 2

# Matmul input dtype. float16: full PE rate (1 cyc/row, FWL hides weight
# loads), half the DMA bytes of fp32, and a 10-bit mantissa (~4x better than
# bf16; fp32 accumulate in PSUM). float32r: fp32 storage but ~2 cyc/row and
# 2x the DMA traffic. bfloat16: same speed as fp16, worse precision.
MM_DTYPE = os.environ.get("DAYMLP_MM_DTYPE", "float16")

_cache: dict = {}
last_run_result = None  # stash of BassKernelResults for test harness use


def _build(mm_dtype_name: str, apply_affine: bool) -> bass.Bass:
    f32 = mybir.dt.float32
    # SBUF tiles feeding the PE carry the matmul dtype directly: for float32r
    # the producing DMA/ACT writes perform the required rounding, for bf16 the
    # DMA/ACT casts. DRAM inputs are plain fp32/bf16 host arrays.
    store_dt = getattr(mybir.dt, mm_dtype_name)
    dram_dt = store_dt

    def mm(ap):
        return ap

    # Bacc (not raw Bass): its compile pipeline moves extra matmul waits onto
    # ldweights and splits >1-wait instructions via event semaphores, which
    # the TRN2 ISA requires.
    nc = bacc.Bacc("TRN2", target_bir_lowering=False)
    # Partition-major DRAM layouts: each SBUF partition's data is one
    # contiguous DRAM run, so every load is 128 large descriptors instead of
    # 128*K small ones (the DMA engines are descriptor-rate limited).
    xt_d = nc.dram_tensor("xt", [S, P, KD, T], dram_dt, kind="ExternalInput")
    w1_d = nc.dram_tensor("w1", [S, P, KD, D_HID], dram_dt, kind="ExternalInput")
    b1_d = nc.dram_tensor("b1", [S, P, KH], f32, kind="ExternalInput")
    w2_d = nc.dram_tensor("w2", [S, P, KH, D_OUT], dram_dt, kind="ExternalInput")
    b2_d = nc.dram_tensor("b2", [S, D_OUT], f32, kind="ExternalInput")
    if apply_affine:
        gm_d = nc.dram_tensor("gm", [S, D_OUT], f32, kind="ExternalInput")
        bt_d = nc.dram_tensor("bt", [S, D_OUT], f32, kind="ExternalInput")
    y_d = nc.dram_tensor("y", [S, T, D_OUT], f32, kind="ExternalOutput")

    with tile.TileContext(nc) as tc:
        with (
            tc.tile_pool(name="xw", bufs=2) as xw,
            tc.tile_pool(name="hb", bufs=2) as hb,
            tc.tile_pool(name="vec", bufs=2) as vec,
            tc.tile_pool(name="yp", bufs=6) as yp,
            tc.tile_pool(name="st", bufs=8) as st,
            tc.tile_pool(name="consts", bufs=1) as cpool,
            tc.tile_pool(name="psum", bufs=8, space="PSUM") as pp,
        ):
            eps_t = cpool.tile([P, 1], f32)
            nc.vector.memset(eps_t, EPS)

            for s in range(S):
                xt_t = xw.tile([P, KD, T], store_dt, tag="xt")
                w1_t = xw.tile([P, KD, D_HID], store_dt, tag="w1")
                if s == 0:
                    # prologue: load xt/w1 per k-chunk so the first matmul
                    # only waits for chunk 0, not the whole 1.5MB
                    for k in range(KD):
                        nc.sync.dma_start(out=xt_t[:, k, :], in_=xt_d[s, :, k, :])
                        nc.sync.dma_start(out=w1_t[:, k, :], in_=w1_d[s, :, k, :])
                else:
                    nc.sync.dma_start(out=xt_t, in_=xt_d[s])
                    nc.sync.dma_start(out=w1_t, in_=w1_d[s])

                def load_rest(s=s):
                    w2_t = xw.tile([P, KH, D_OUT], store_dt, tag="w2")
                    nc.sync.dma_start(out=w2_t, in_=w2_d[s])
                    b2_t = vec.tile([P, 1, D_OUT], f32, tag="b2")
                    nc.gpsimd.dma_start(
                        out=b2_t, in_=b2_d[s : s + 1, :].partition_broadcast(P)
                    )
                    gm_t = bt_t = None
                    if apply_affine:
                        gm_t = vec.tile([P, 1, D_OUT], f32, tag="gm")
                        nc.gpsimd.dma_start(
                            out=gm_t, in_=gm_d[s : s + 1, :].partition_broadcast(P)
                        )
                        bt_t = vec.tile([P, 1, D_OUT], f32, tag="bt")
                        nc.gpsimd.dma_start(
                            out=bt_t, in_=bt_d[s : s + 1, :].partition_broadcast(P)
                        )
                    return w2_t, b2_t, gm_t, bt_t

                b1_t = vec.tile([P, KH], f32, tag="b1")
                if s > 0:
                    # pass-2 operands up front so DMA overlaps pass-1 compute
                    w2_t, b2_t, gm_t, bt_t = load_rest()
                    nc.sync.dma_start(out=b1_t, in_=b1_d[s])

                # pass 1: hT[h, :] = relu(W1[:, h].T @ xT + b1[h])
                hT_t = hb.tile([P, KH, T], store_dt, tag="hT")
                if s == 0:
                    # k-outer over all 8 PSUM banks: matmuls start as soon as
                    # chunk k=0 has landed
                    ps_list = [pp.tile([P, T], f32, tag="ps", name=f"ps0_{h}") for h in range(KH)]
                    for k in range(KD):
                        for h in range(KH):
                            nc.tensor.matmul(
                                ps_list[h],
                                lhsT=mm(w1_t[:, k, P * h : P * (h + 1)]),
                                rhs=mm(xt_t[:, k, :]),
                                start=(k == 0),
                                stop=(k == KD - 1),
                            )
                    nc.sync.dma_start(out=b1_t, in_=b1_d[s])
                    w2_t, b2_t, gm_t, bt_t = load_rest()
                    for h in range(KH):
                        nc.scalar.activation(
                            out=hT_t[:, h, :],
                            in_=ps_list[h],
                            func=mybir.ActivationFunctionType.Relu,
                            bias=b1_t[:, h : h + 1],
                            scale=1.0,
                        )
                else:
                    for h in range(KH):
                        ps = pp.tile([P, T], f32, tag="ps")
                        for k in range(KD):
                            nc.tensor.matmul(
                                ps,
                                lhsT=mm(w1_t[:, k, P * h : P * (h + 1)]),
                                rhs=mm(xt_t[:, k, :]),
                                start=(k == 0),
                                stop=(k == KD - 1),
                            )
                        nc.scalar.activation(
                            out=hT_t[:, h, :],
                            in_=ps,
                            func=mybir.ActivationFunctionType.Relu,
                            bias=b1_t[:, h : h + 1],
                            scale=1.0,
                        )

                # pass 2: y[t_tile, :] = hT[:, t_tile].T @ W2 (+ b2), then LN
                for t in range(MT):
                    ps2 = pp.tile([P, D_OUT], f32, tag="ps")
                    for k in range(KH):
                        nc.tensor.matmul(
                            ps2,
                            lhsT=mm(hT_t[:, k, P * t : P * (t + 1)]),
                            rhs=mm(w2_t[:, k, :]),
                            start=(k == 0),
                            stop=(k == KH - 1),
                        )
                    y_t = yp.tile([P, D_OUT], f32, tag="y")
                    nc.vector.tensor_add(out=y_t, in0=ps2, in1=b2_t[:, 0, :])
                    stats = st.tile([P, 6], f32, tag="stats")
                    nc.vector.bn_stats(out=stats, in_=y_t)
                    mv = st.tile([P, 2], f32, tag="mv")
                    nc.vector.bn_aggr(out=mv, in_=stats)
                    rstd = st.tile([P, 1], f32, tag="rstd")
                    nc.scalar.activation(
                        out=rstd,
                        in_=mv[:, 1:2],
                        func=mybir.ActivationFunctionType.Sqrt,
                        bias=eps_t,
                        scale=1.0,
                    )
                    nc.vector.reciprocal(out=rstd, in_=rstd)
                    # normalize on GpSimd: frees DVE (busy with bn_stats and
                    # the b2 add) and shortens the per-tile dependency chain
                    nc.gpsimd.tensor_scalar(
                        out=y_t,
                        in0=y_t,
                        scalar1=mv[:, 0:1],
                        scalar2=rstd,
                        op0=mybir.AluOpType.subtract,
                        op1=mybir.AluOpType.mult,
                    )
                    if apply_affine:
                        nc.vector.tensor_mul(out=y_t, in0=y_t, in1=gm_t[:, 0, :])
                        nc.vector.tensor_add(out=y_t, in0=y_t, in1=bt_t[:, 0, :])
                    nc.sync.dma_start(out=y_d[s, P * t : P * (t + 1), :], in_=y_t)
    nc.finalize()
    return nc


def kernel(**inputs) -> np.ndarray:
    global last_run_result
    x = np.asarray(inputs["x"], dtype=np.float32)
    day = np.asarray(inputs["day_indices"]).astype(np.int64)
    W1 = np.asarray(inputs["W1"], dtype=np.float32)
    b1 = np.asarray(inputs["b1"], dtype=np.float32)
    W2 = np.asarray(inputs["W2"], dtype=np.float32)
    b2 = np.asarray(inputs["b2"], dtype=np.float32)
    gamma = np.asarray(inputs["gamma"], dtype=np.float32)
    beta = np.asarray(inputs["beta"], dtype=np.float32)

    apply_affine = not (np.all(gamma == 1.0) and np.all(beta == 0.0))
    key = (MM_DTYPE, apply_affine)
    if key not in _cache:
        _cache[key] = _build(*key)
    nc = _cache[key]

    mm_np = {
        "bfloat16": ml_dtypes.bfloat16,
        "float16": np.float16,
    }.get(MM_DTYPE, np.float32)

    # host-side routing gather + layout prep: K on partitions, and
    # partition-major so each partition's DMA data is contiguous in DRAM
    xt = np.ascontiguousarray(
        x.transpose(0, 2, 1).reshape(B, KD, P, T).transpose(0, 2, 1, 3).astype(mm_np)
    )
    W1d = np.ascontiguousarray(
        W1[day].reshape(B, KD, P, D_HID).transpose(0, 2, 1, 3).astype(mm_np)
    )
    W2d = np.ascontiguousarray(
        W2[day].reshape(B, KH, P, D_OUT).transpose(0, 2, 1, 3).astype(mm_np)
    )
    b1d = np.ascontiguousarray(b1[day].reshape(B, KH, P).transpose(0, 2, 1))
    b2d = np.ascontiguousarray(b2[day])
    gmd = np.ascontiguousarray(gamma[day])
    btd = np.ascontiguousarray(beta[day])

    in_maps = []
    for c in range(N_CORES):
        sl = slice(c * S, (c + 1) * S)
        m = {
            "xt": xt[sl],
            "w1": W1d[sl],
            "b1": b1d[sl],
            "w2": W2d[sl],
            "b2": b2d[sl],
        }
        if apply_affine:
            m["gm"] = gmd[sl]
            m["bt"] = btd[sl]
        in_maps.append(m)

    trace = os.environ.get("DAYMLP_TRACE", "0") == "1"
    res = run_bass_kernel_spmd(
        nc,
        in_maps,
        core_ids=list(range(N_CORES)),
        trace=trace,
    )
    last_run_result = res
    y = np.concatenate([r["y"] for r in res.results], axis=0)
    return y.astype(np.float32)


# revision 14
# speedup vs baseline: 2.0681x; 2.0681x over previous
"""Day-routed adapter MLP (per-sample day-specific 2-layer MLP + LayerNorm)
for 8 Trainium2 NeuronCores.

Computation per sample b (day d = day_indices[b]):
    h = relu(x[b] @ W1[d] + b1[d])        # [T, D_hid]
    y = h @ W2[d] + b2[d]                 # [T, D_out]
    out = LN(y) * gamma[d] + beta[d]      # LN over last dim

Sharding: data-parallel over batch, 8 samples per core. The per-sample
day weights are gathered on the host (routing is host-visible), and x is
pre-transposed on the host so the device needs no transposes at all:

  pass 1:  hT[h_chunk, :T] += W1[k_chunk, h_chunk].T @ xT[k_chunk, :T]
           (lhsT = W1 natural layout, rhs = xT)  -> hT with H on partitions,
           so b1 is a per-partition bias fused into the ReLU copyback (ACT).
  pass 2:  y[t_tile, :O]  += hT[k_chunk, t_tile].T @ W2[k_chunk, :O]
           (lhsT = hT from pass 1, rhs = W2 natural layout) -> y with T on
           partitions and O on the free axis, which is exactly the layout
           LayerNorm wants (bn_stats/bn_aggr reduce along free axis).
"""

import os

import numpy as np
import ml_dtypes

import concourse.bass as bass
import concourse.mybir as mybir
import concourse.tile as tile
from concourse import bacc
from concourse.bass_utils import run_bass_kernel_spmd

N_CORES = 8
B, T, D_IN = 64, 512, 512
D_HID, D_OUT = 1024, 512
S = B // N_CORES  # samples per core
EPS = 1e-5

P = 128
KD = D_IN // P   # 4 contraction chunks in pass 1
KH = D_HID // P  # 8 contraction chunks in pass 2 (= H chunks of pass 1 out)
MT = T // P      # 4 token tiles in pass 2

# Matmul input dtype. float16: full PE rate (1 cyc/row, FWL hides weight
# loads), half the DMA bytes of fp32, and a 10-bit mantissa (~4x better than
# bf16; fp32 accumulate in PSUM). float32r: fp32 storage but ~2 cyc/row and
# 2x the DMA traffic. bfloat16: same speed as fp16, worse precision.
MM_DTYPE = os.environ.get("DAYMLP_MM_DTYPE", "float16")

_cache: dict = {}
last_run_result = None  # stash of BassKernelResults for test harness use


def _build(mm_dtype_name: str, apply_affine: bool) -> bass.Bass:
    f32 = mybir.dt.float32
    # SBUF tiles feeding the PE carry the matmul dtype directly: for float32r
    # the producing DMA/ACT writes perform the required rounding, for bf16 the
    # DMA/ACT casts. DRAM inputs are plain fp32/bf16 host arrays.
    store_dt = getattr(mybir.dt, mm_dtype_name)
    dram_dt = store_dt

    def mm(ap):
        return ap

    # Bacc (not raw Bass): its compile pipeline moves extra matmul waits onto
    # ldweights and splits >1-wait instructions via event semaphores, which
    # the TRN2 ISA requires.
    nc = bacc.Bacc("TRN2", target_bir_lowering=False)
    # Partition-major DRAM layouts: each SBUF partition's data is one
    # contiguous DRAM run, so every load is 128 large descriptors instead of
    # 128*K small ones (the DMA engines are descriptor-rate limited).
    xt_d = nc.dram_tensor("xt", [S, P, KD, T], dram_dt, kind="ExternalInput")
    w1_d = nc.dram_tensor("w1", [S, P, KD, D_HID], dram_dt, kind="ExternalInput")
    b1_d = nc.dram_tensor("b1", [S, P, KH], f32, kind="ExternalInput")
    w2_d = nc.dram_tensor("w2", [S, P, KH, D_OUT], dram_dt, kind="ExternalInput")
    b2_d = nc.dram_tensor("b2", [S, D_OUT], f32, kind="ExternalInput")
    if apply_affine:
        gm_d = nc.dram_tensor("gm", [S, D_OUT], f32, kind="ExternalInput")
        bt_d = nc.dram_tensor("bt", [S, D_OUT], f32, kind="ExternalInput")
    y_d = nc.dram_tensor("y", [S, T, D_OUT], f32, kind="ExternalOutput")

    with tile.TileContext(nc) as tc:
        with (
            tc.tile_pool(name="xw", bufs=2) as xw,
            tc.tile_pool(name="hb", bufs=2) as hb,
            tc.tile_pool(name="vec", bufs=2) as vec,
            tc.tile_pool(name="yp", bufs=6) as yp,
            tc.tile_pool(name="st", bufs=8) as st,
            tc.tile_pool(name="consts", bufs=1) as cpool,
            tc.tile_pool(name="psum", bufs=8, space="PSUM") as pp,
        ):
            eps_t = cpool.tile([P, 1], f32)
            nc.vector.memset(eps_t, EPS)

            for s in range(S):
                xt_t = xw.tile([P, KD, T], store_dt, tag="xt")
                w1_t = xw.tile([P, KD, D_HID], store_dt, tag="w1")
                if s == 0:
                    # prologue: load xt/w1 per k-chunk so the first matmul
                    # only waits for chunk 0, not the whole 1.5MB
                    for k in range(KD):
                        nc.sync.dma_start(out=xt_t[:, k, :], in_=xt_d[s, :, k, :])
                        nc.sync.dma_start(out=w1_t[:, k, :], in_=w1_d[s, :, k, :])
                else:
                    nc.sync.dma_start(out=xt_t, in_=xt_d[s])
                    nc.sync.dma_start(out=w1_t, in_=w1_d[s])

                def load_rest(s=s):
                    w2_t = xw.tile([P, KH, D_OUT], store_dt, tag="w2")
                    nc.sync.dma_start(out=w2_t, in_=w2_d[s])
                    b2_t = vec.tile([P, 1, D_OUT], f32, tag="b2")
                    nc.gpsimd.dma_start(
                        out=b2_t, in_=b2_d[s : s + 1, :].partition_broadcast(P)
                    )
                    gm_t = bt_t = None
                    if apply_affine:
                        gm_t = vec.tile([P, 1, D_OUT], f32, tag="gm")
                        nc.gpsimd.dma_start(
                            out=gm_t, in_=gm_d[s : s + 1, :].partition_broadcast(P)
                        )
                        bt_t = vec.tile([P, 1, D_OUT], f32, tag="bt")
                        nc.gpsimd.dma_start(
                            out=bt_t, in_=bt_d[s : s + 1, :].partition_broadcast(P)
                        )
                    return w2_t, b2_t, gm_t, bt_t

                b1_t = vec.tile([P, KH], f32, tag="b1")
                if s > 0:
                    # pass-2 operands up front so DMA overlaps pass-1 compute
                    w2_t, b2_t, gm_t, bt_t = load_rest()
                    nc.sync.dma_start(out=b1_t, in_=b1_d[s])

                # pass 1: hT[h, :] = relu(W1[:, h].T @ xT + b1[h])
                hT_t = hb.tile([P, KH, T], store_dt, tag="hT")
                if s == 0:
                    # k-outer over all 8 PSUM banks: matmuls start as soon as
                    # chunk k=0 has landed
                    ps_list = [pp.tile([P, T], f32, tag="ps", name=f"ps0_{h}") for h in range(KH)]
                    for k in range(KD):
                        for h in range(KH):
                            nc.tensor.matmul(
                                ps_list[h],
                                lhsT=mm(w1_t[:, k, P * h : P * (h + 1)]),
                                rhs=mm(xt_t[:, k, :]),
                                start=(k == 0),
                                stop=(k == KD - 1),
                            )
                    nc.sync.dma_start(out=b1_t, in_=b1_d[s])
                    w2_t, b2_t, gm_t, bt_t = load_rest()
                    for h in range(KH):
                        nc.scalar.activation(
                            out=hT_t[:, h, :],
                            in_=ps_list[h],
                            func=mybir.ActivationFunctionType.Relu,
                            bias=b1_t[:, h : h + 1],
                            scale=1.0,
                        )
                else:
                    for h in range(KH):
                        ps = pp.tile([P, T], f32, tag="ps")
                        for k in range(KD):
                            nc.tensor.matmul(
                                ps,
                                lhsT=mm(w1_t[:, k, P * h : P * (h + 1)]),
                                rhs=mm(xt_t[:, k, :]),
                                start=(k == 0),
                                stop=(k == KD - 1),
                            )
                        nc.scalar.activation(
                            out=hT_t[:, h, :],
                            in_=ps,
                            func=mybir.ActivationFunctionType.Relu,
                            bias=b1_t[:, h : h + 1],
                            scale=1.0,
                        )

                # pass 2: y[t_tile, :] = hT[:, t_tile].T @ W2 (+ b2), then LN
                for t in range(MT):
                    ps2 = pp.tile([P, D_OUT], f32, tag="ps")
                    for k in range(KH):
                        nc.tensor.matmul(
                            ps2,
                            lhsT=mm(hT_t[:, k, P * t : P * (t + 1)]),
                            rhs=mm(w2_t[:, k, :]),
                            start=(k == 0),
                            stop=(k == KH - 1),
                        )
                    y_t = yp.tile([P, D_OUT], f32, tag="y")
                    nc.vector.tensor_add(out=y_t, in0=ps2, in1=b2_t[:, 0, :])
                    stats = st.tile([P, 6], f32, tag="stats")
                    nc.vector.bn_stats(out=stats, in_=y_t)
                    mv = st.tile([P, 2], f32, tag="mv")
                    nc.vector.bn_aggr(out=mv, in_=stats)
                    rstd = st.tile([P, 1], f32, tag="rstd")
                    nc.scalar.activation(
                        out=rstd,
                        in_=mv[:, 1:2],
                        func=mybir.ActivationFunctionType.Sqrt,
                        bias=eps_t,
                        scale=1.0,
                    )
                    nc.vector.reciprocal(out=rstd, in_=rstd)
                    nc.vector.tensor_scalar(
                        out=y_t,
                        in0=y_t,
                        scalar1=mv[:, 0:1],
                        scalar2=rstd,
                        op0=mybir.AluOpType.subtract,
                        op1=mybir.AluOpType.mult,
                    )
                    if apply_affine:
                        nc.vector.tensor_mul(out=y_t, in0=y_t, in1=gm_t[:, 0, :])
                        nc.vector.tensor_add(out=y_t, in0=y_t, in1=bt_t[:, 0, :])
                    nc.sync.dma_start(out=y_d[s, P * t : P * (t + 1), :], in_=y_t)
    nc.finalize()
    return nc


def kernel(**inputs) -> np.ndarray:
    global last_run_result
    x = np.asarray(inputs["x"], dtype=np.float32)
    day = np.asarray(inputs["day_indices"]).astype(np.int64)
    W1 = np.asarray(inputs["W1"], dtype=np.float32)
    b1 = np.asarray(inputs["b1"], dtype=np.float32)
    W2 = np.asarray(inputs["W2"], dtype=np.float32)
    b2 = np.asarray(inputs["b2"], dtype=np.float32)
    gamma = np.asarray(inputs["gamma"], dtype=np.float32)
    beta = np.asarray(inputs["beta"], dtype=np.float32)

    apply_affine = not (np.all(gamma == 1.0) and np.all(beta == 0.0))
    key = (MM_DTYPE, apply_affine)
    if key not in _cache:
        _cache[key] = _build(*key)
    nc = _cache[key]

    mm_np = {
        "bfloat16": ml_dtypes.bfloat16,
        "float16": np.float16,
    }.get(MM_DTYPE, np.float32)

    # host-side routing gather + layout prep: K on partitions, and
    # partition-major so each partition's DMA data is contiguous in DRAM
    xt = np.ascontiguousarray(
        x.transpose(0, 2, 1).reshape(B, KD, P, T).transpose(0, 2, 1, 3).astype(mm_np)
    )
    W1d = np.ascontiguousarray(
        W1[day].reshape(B, KD, P, D_HID).transpose(0, 2, 1, 3).astype(mm_np)
    )
    W2d = np.ascontiguousarray(
        W2[day].reshape(B, KH, P, D_OUT).transpose(0, 2, 1, 3).astype(mm_np)
    )
    b1d = np.ascontiguousarray(b1[day].reshape(B, KH, P).transpose(0, 2, 1))
    b2d = np.ascontiguousarray(b2[day])
    gmd = np.ascontiguousarray(gamma[day])
    btd = np.ascontiguousarray(beta[day])

    in_maps = []
    for c in range(N_CORES):
        sl = slice(c * S, (c + 1) * S)
        m = {
            "xt": xt[sl],
            "w1": W1d[sl],
            "b1": b1d[sl],
            "w2": W2d[sl],
            "b2": b2d[sl],
        }
        if apply_affine:
            m["gm"] = gmd[sl]
            m["bt"] = btd[sl]
        in_maps.append(m)

    trace = os.environ.get("DAYMLP_TRACE", "0") == "1"
    res = run_bass_kernel_spmd(
        nc,
        in_maps,
        core_ids=list(range(N_CORES)),
        trace=trace,
    )
    last_run_result = res
    y = np.concatenate([r["y"] for r in res.results], axis=0)
    return y.astype(np.float32)


# revision 17
# speedup vs baseline: 2.0846x; 1.0080x over previous
"""Day-routed adapter MLP (per-sample day-specific 2-layer MLP + LayerNorm)
for 8 Trainium2 NeuronCores.

Computation per sample b (day d = day_indices[b]):
    h = relu(x[b] @ W1[d] + b1[d])        # [T, D_hid]
    y = h @ W2[d] + b2[d]                 # [T, D_out]
    out = LN(y) * gamma[d] + beta[d]      # LN over last dim

Sharding: data-parallel over batch, 8 samples per core. The per-sample
day weights are gathered on the host (routing is host-visible), and x is
pre-transposed on the host so the device needs no transposes at all:

  pass 1:  hT[h_chunk, :T] += W1[k_chunk, h_chunk].T @ xT[k_chunk, :T]
           (lhsT = W1 natural layout, rhs = xT)  -> hT with H on partitions,
           so b1 is a per-partition bias fused into the ReLU copyback (ACT).
  pass 2:  y[t_tile, :O]  += hT[k_chunk, t_tile].T @ W2[k_chunk, :O]
           (lhsT = hT from pass 1, rhs = W2 natural layout) -> y with T on
           partitions and O on the free axis, which is exactly the layout
           LayerNorm wants (bn_stats/bn_aggr reduce along free axis).
"""

import os

import numpy as np
import ml_dtypes

import concourse.bass as bass
import concourse.mybir as mybir
import concourse.tile as tile
from concourse import bacc
from concourse.bass_utils import run_bass_kernel_spmd

N_CORES = 8
B, T, D_IN = 64, 512, 512
D_HID, D_OUT = 1024, 512
S = B // N_CORES  # samples per core
EPS = 1e-5

P = 128
KD = D_IN // P   # 4 contraction chunks in pass 1
KH = D_HID // P  # 8 contraction chunks in pass 2 (= H chunks of pass 1 out)
MT = T // P      # 4 token tiles in pass 2

# Matmul input dtype. float16: full PE rate (1 cyc/row, FWL hides weight
# loads), half the DMA bytes of fp32, and a 10-bit mantissa (~4x better than
# bf16; fp32 accumulate in PSUM). float32r: fp32 storage but ~2 cyc/row and
# 2x the DMA traffic. bfloat16: same speed as fp16, worse precision.
MM_DTYPE = os.environ.get("DAYMLP_MM_DTYPE", "float16")

_cache: dict = {}
last_run_result = None  # stash of BassKernelResults for test harness use


def _build(mm_dtype_name: str, apply_affine: bool) -> bass.Bass:
    f32 = mybir.dt.float32
    # SBUF tiles feeding the PE carry the matmul dtype directly: for float32r
    # the producing DMA/ACT writes perform the required rounding, for bf16 the
    # DMA/ACT casts. DRAM inputs are plain fp32/bf16 host arrays.
    store_dt = getattr(mybir.dt, mm_dtype_name)
    dram_dt = store_dt

    def mm(ap):
        return ap

    # Bacc (not raw Bass): its compile pipeline moves extra matmul waits onto
    # ldweights and splits >1-wait instructions via event semaphores, which
    # the TRN2 ISA requires.
    nc = bacc.Bacc("TRN2", target_bir_lowering=False)
    # Partition-major DRAM layouts: each SBUF partition's data is one
    # contiguous DRAM run, so every load is 128 large descriptors instead of
    # 128*K small ones (the DMA engines are descriptor-rate limited).
    xt_d = nc.dram_tensor("xt", [S, P, KD, T], dram_dt, kind="ExternalInput")
    w1_d = nc.dram_tensor("w1", [S, P, KD, D_HID], dram_dt, kind="ExternalInput")
    b1_d = nc.dram_tensor("b1", [S, P, KH], f32, kind="ExternalInput")
    w2_d = nc.dram_tensor("w2", [S, P, KH, D_OUT], dram_dt, kind="ExternalInput")
    b2_d = nc.dram_tensor("b2", [S, D_OUT], f32, kind="ExternalInput")
    if apply_affine:
        gm_d = nc.dram_tensor("gm", [S, D_OUT], f32, kind="ExternalInput")
        bt_d = nc.dram_tensor("bt", [S, D_OUT], f32, kind="ExternalInput")
    y_d = nc.dram_tensor("y", [S, T, D_OUT], f32, kind="ExternalOutput")

    with tile.TileContext(nc) as tc:
        with (
            tc.tile_pool(name="xw", bufs=2) as xw,
            tc.tile_pool(name="hb", bufs=2) as hb,
            tc.tile_pool(name="vec", bufs=2) as vec,
            tc.tile_pool(name="yp", bufs=6) as yp,
            tc.tile_pool(name="st", bufs=8) as st,
            tc.tile_pool(name="consts", bufs=1) as cpool,
            tc.tile_pool(name="prologue", bufs=1) as pro,
            tc.tile_pool(name="psum", bufs=8, space="PSUM") as pp,
        ):
            eps_t = cpool.tile([P, 1], f32)
            nc.vector.memset(eps_t, EPS)

            for s in range(S):
                if s == 0:
                    # prologue: per-k-chunk tiles with loads split across two
                    # issuing engines, so the first matmul waits only for
                    # chunk 0 of xt/w1 (~0.4MB), not the whole 1.5MB
                    xt_ck = []
                    w1_ck = []
                    for k in range(KD):
                        xt_c = pro.tile([P, T], store_dt, tag=f"xt0_{k}", name=f"xt0_{k}")
                        nc.sync.dma_start(out=xt_c, in_=xt_d[s, :, k, :])
                        xt_ck.append(xt_c)
                        w1_c = pro.tile([P, D_HID], store_dt, tag=f"w10_{k}", name=f"w10_{k}")
                        nc.gpsimd.dma_start(out=w1_c, in_=w1_d[s, :, k, :])
                        w1_ck.append(w1_c)
                else:
                    xt_t = xw.tile([P, KD, T], store_dt, tag="xt")
                    w1_t = xw.tile([P, KD, D_HID], store_dt, tag="w1")
                    nc.sync.dma_start(out=xt_t, in_=xt_d[s])
                    nc.sync.dma_start(out=w1_t, in_=w1_d[s])

                def load_rest(s=s):
                    w2_t = xw.tile([P, KH, D_OUT], store_dt, tag="w2")
                    nc.sync.dma_start(out=w2_t, in_=w2_d[s])
                    b2_t = vec.tile([P, 1, D_OUT], f32, tag="b2")
                    nc.gpsimd.dma_start(
                        out=b2_t, in_=b2_d[s : s + 1, :].partition_broadcast(P)
                    )
                    gm_t = bt_t = None
                    if apply_affine:
                        gm_t = vec.tile([P, 1, D_OUT], f32, tag="gm")
                        nc.gpsimd.dma_start(
                            out=gm_t, in_=gm_d[s : s + 1, :].partition_broadcast(P)
                        )
                        bt_t = vec.tile([P, 1, D_OUT], f32, tag="bt")
                        nc.gpsimd.dma_start(
                            out=bt_t, in_=bt_d[s : s + 1, :].partition_broadcast(P)
                        )
                    return w2_t, b2_t, gm_t, bt_t

                b1_t = vec.tile([P, KH], f32, tag="b1")
                if s > 0:
                    # pass-2 operands up front so DMA overlaps pass-1 compute
                    w2_t, b2_t, gm_t, bt_t = load_rest()
                    nc.sync.dma_start(out=b1_t, in_=b1_d[s])

                # pass 1: hT[h, :] = relu(W1[:, h].T @ xT + b1[h])
                hT_t = hb.tile([P, KH, T], store_dt, tag="hT")
                if s == 0:
                    # k-outer over all 8 PSUM banks: matmuls start as soon as
                    # chunk k=0 has landed
                    ps_list = [pp.tile([P, T], f32, tag="ps", name=f"ps0_{h}") for h in range(KH)]
                    for k in range(KD):
                        for h in range(KH):
                            nc.tensor.matmul(
                                ps_list[h],
                                lhsT=mm(w1_ck[k][:, P * h : P * (h + 1)]),
                                rhs=mm(xt_ck[k]),
                                start=(k == 0),
                                stop=(k == KD - 1),
                            )
                    nc.sync.dma_start(out=b1_t, in_=b1_d[s])
                    w2_t, b2_t, gm_t, bt_t = load_rest()
                    for h in range(KH):
                        nc.scalar.activation(
                            out=hT_t[:, h, :],
                            in_=ps_list[h],
                            func=mybir.ActivationFunctionType.Relu,
                            bias=b1_t[:, h : h + 1],
                            scale=1.0,
                        )
                else:
                    for h in range(KH):
                        ps = pp.tile([P, T], f32, tag="ps")
                        for k in range(KD):
                            nc.tensor.matmul(
                                ps,
                                lhsT=mm(w1_t[:, k, P * h : P * (h + 1)]),
                                rhs=mm(xt_t[:, k, :]),
                                start=(k == 0),
                                stop=(k == KD - 1),
                            )
                        nc.scalar.activation(
                            out=hT_t[:, h, :],
                            in_=ps,
                            func=mybir.ActivationFunctionType.Relu,
                            bias=b1_t[:, h : h + 1],
                            scale=1.0,
                        )

                # pass 2: y[t_tile, :] = hT[:, t_tile].T @ W2 (+ b2), then LN
                for t in range(MT):
                    ps2 = pp.tile([P, D_OUT], f32, tag="ps")
                    for k in range(KH):
                        nc.tensor.matmul(
                            ps2,
                            lhsT=mm(hT_t[:, k, P * t : P * (t + 1)]),
                            rhs=mm(w2_t[:, k, :]),
                            start=(k == 0),
                            stop=(k == KH - 1),
                        )
                    y_t = yp.tile([P, D_OUT], f32, tag="y")
                    nc.vector.tensor_add(out=y_t, in0=ps2, in1=b2_t[:, 0, :])
                    stats = st.tile([P, 6], f32, tag="stats")
                    nc.vector.bn_stats(out=stats, in_=y_t)
                    mv = st.tile([P, 2], f32, tag="mv")
                    nc.vector.bn_aggr(out=mv, in_=stats)
                    rstd = st.tile([P, 1], f32, tag="rstd")
                    nc.scalar.activation(
                        out=rstd,
                        in_=mv[:, 1:2],
                        func=mybir.ActivationFunctionType.Sqrt,
                        bias=eps_t,
                        scale=1.0,
                    )
                    nc.vector.reciprocal(out=rstd, in_=rstd)
                    nc.vector.tensor_scalar(
                        out=y_t,
                        in0=y_t,
                        scalar1=mv[:, 0:1],
                        scalar2=rstd,
                        op0=mybir.AluOpType.subtract,
                        op1=mybir.AluOpType.mult,
                    )
                    if apply_affine:
                        nc.vector.tensor_mul(out=y_t, in0=y_t, in1=gm_t[:, 0, :])
                        nc.vector.tensor_add(out=y_t, in0=y_t, in1=bt_t[:, 0, :])
                    nc.sync.dma_start(out=y_d[s, P * t : P * (t + 1), :], in_=y_t)
    nc.finalize()
    return nc


def kernel(**inputs) -> np.ndarray:
    global last_run_result
    x = np.asarray(inputs["x"], dtype=np.float32)
    day = np.asarray(inputs["day_indices"]).astype(np.int64)
    W1 = np.asarray(inputs["W1"], dtype=np.float32)
    b1 = np.asarray(inputs["b1"], dtype=np.float32)
    W2 = np.asarray(inputs["W2"], dtype=np.float32)
    b2 = np.asarray(inputs["b2"], dtype=np.float32)
    gamma = np.asarray(inputs["gamma"], dtype=np.float32)
    beta = np.asarray(inputs["beta"], dtype=np.float32)

    apply_affine = not (np.all(gamma == 1.0) and np.all(beta == 0.0))
    key = (MM_DTYPE, apply_affine)
    if key not in _cache:
        _cache[key] = _build(*key)
    nc = _cache[key]

    mm_np = {
        "bfloat16": ml_dtypes.bfloat16,
        "float16": np.float16,
    }.get(MM_DTYPE, np.float32)

    # host-side routing gather + layout prep: K on partitions, and
    # partition-major so each partition's DMA data is contiguous in DRAM
    xt = np.ascontiguousarray(
        x.transpose(0, 2, 1).reshape(B, KD, P, T).transpose(0, 2, 1, 3).astype(mm_np)
    )
    W1d = np.ascontiguousarray(
        W1[day].reshape(B, KD, P, D_HID).transpose(0, 2, 1, 3).astype(mm_np)
    )
    W2d = np.ascontiguousarray(
        W2[day].reshape(B, KH, P, D_OUT).transpose(0, 2, 1, 3).astype(mm_np)
    )
    b1d = np.ascontiguousarray(b1[day].reshape(B, KH, P).transpose(0, 2, 1))
    b2d = np.ascontiguousarray(b2[day])
    gmd = np.ascontiguousarray(gamma[day])
    btd = np.ascontiguousarray(beta[day])

    in_maps = []
    for c in range(N_CORES):
        sl = slice(c * S, (c + 1) * S)
        m = {
            "xt": xt[sl],
            "w1": W1d[sl],
            "b1": b1d[sl],
            "w2": W2d[sl],
            "b2": b2d[sl],
        }
        if apply_affine:
            m["gm"] = gmd[sl]
            m["bt"] = btd[sl]
        in_maps.append(m)

    trace = os.environ.get("DAYMLP_TRACE", "0") == "1"
    res = run_bass_kernel_spmd(
        nc,
        in_maps,
        core_ids=list(range(N_CORES)),
        trace=trace,
    )
    last_run_result = res
    y = np.concatenate([r["y"] for r in res.results], axis=0)
    return y.astype(np.float32)
